# revision 59
# baseline (speedup 1.0000x reference)
"""Trainium2 Bass kernel for nn_Attention_50405736186248.

Module: out = (softmax(causal(rope(q@Wq.T) @ rope(k@Wk.T).T / sqrt(128)))
               @ vh) @ Wo.T
where vh = concat([v[:,:,512:1024] @ Wv.T, 0, 0], -1).

Structure exploited: vh is zero for heads 4..11, so only heads 0..3
contribute to the output.  That gives 2 batches x 4 live heads = 8
independent units -> one per NeuronCore, no collectives.  Host sums the
4 per-head partial out-projections per batch at gather time.

RoPE handling: the interleaved-pair rotation equals a rotate-half
rotation after a fixed permutation P of the head dim; P is folded into
Wq/Wk on the host.  The cross-half swap J is done as a 128x128
exchange-matrix matmul on the PE; cos/sin (sign-folded) multiplies run
on the DVE over [128 partitions, seq] tiles.

Softmax: scores are bounded (|s| <~ 7), so exp without max-subtraction
is safe in fp32.  Row sums come from a ones-vector matmul accumulated in
PSUM; normalization is applied as a per-partition scale during the final
PSUM->SBUF copy of the out-projection.

The program is a per-seq-chunk software pipeline:
  for c in 0..3: [dma q/k chunk] -> [proj+rope q/k] -> [vmid tiles] ->
                 [attention c] -> [recip c] -> [outproj c]
so Tile can overlap chunk c+1's DMA/projections with chunk c's
attention/out-projection.
"""

import math
import os
import sys

import numpy as np

sys.path.insert(0, "/opt/trn_rl_repo")

import ml_dtypes  # noqa: E402

import concourse.bacc as bacc  # noqa: E402
import concourse.mybir as mybir  # noqa: E402
import concourse.tile as tile  # noqa: E402
from concourse import bass_utils  # noqa: E402

B = 2
S = 2048
D = 1536
N_HEADS_LIVE = 4  # heads with nonzero V
HD = 128  # head dim
D_AUG = 512
ROPE_BASE = 10000.0

SC = 512  # seq chunk (PSUM free dim)
NSC = S // SC  # 4
NE = D // 128  # 12 embed chunks
NA = D_AUG // 128  # 4 aug chunks
NT = S // 128  # 16 seq tiles
NIC = D // SC  # 3 out-proj column chunks

F32 = mybir.dt.float32
BF16 = mybir.dt.bfloat16
BF = ml_dtypes.bfloat16

OUT_DTYPE = BF16  # partial outputs; host sums in fp32
OUT_NP = BF

EXP = mybir.ActivationFunctionType.Exp
COPY = mybir.ActivationFunctionType.Copy


def _build_program(reps=1):
    nc = bacc.Bacc("TRN2", target_bir_lowering=False, debug=False, num_devices=8)

    def din(name, shape, dt):
        return nc.dram_tensor(name, shape, dt, kind="ExternalInput").ap()

    qT = din("qT", [D, S], BF16)
    kT = din("kT", [D, S], BF16)
    vT = din("vT", [128, NA, S], BF16)  # host pre-arranged (p, a, s)
    wq = din("wq", [128, NE, HD], BF16)  # host pre-arranged (p, e, m)
    wk = din("wk", [128, NE, HD], BF16)
    wv = din("wv", [128, NA, HD], BF16)
    wo = din("wo", [HD, D], BF16)
    cosT = din("cosT", [128, S], BF16)
    sinT = din("sinT", [128, S], BF16)
    jmat = din("jmat", [128, 128], BF16)
    onesv = din("onesv", [128, 1], BF16)
    tri = din("tri", [128, 128], BF16)
    out = nc.dram_tensor("out", [S, D], OUT_DTYPE, kind="ExternalOutput").ap()

    inv_sqrt_d = 1.0 / math.sqrt(HD)

    with tile.TileContext(nc) as tc:
        with (
            tc.tile_pool(name="const", bufs=1) as cpool,
            tc.tile_pool(name="qstream", bufs=2) as qpool,
            tc.tile_pool(name="kstream", bufs=2) as kpool,
            tc.tile_pool(name="rope", bufs=1) as rpool,
            tc.tile_pool(name="raw", bufs=4) as rawpool,
            tc.tile_pool(name="tmp", bufs=4) as tmppool,
            tc.tile_pool(name="norm", bufs=2) as npool,
            tc.tile_pool(name="probs", bufs=6) as prpool,
            tc.tile_pool(name="outsb", bufs=16) as opool,
            tc.tile_pool(name="small", bufs=1) as spool,
            tc.tile_pool(name="ps_mm", bufs=2, space="PSUM") as ps_mm,
            tc.tile_pool(name="ps_sc", bufs=2, space="PSUM") as ps_sc,
            tc.tile_pool(name="ps_at", bufs=1, space="PSUM") as ps_at,
            tc.tile_pool(name="ps_sum", bufs=1, space="PSUM") as ps_sum,
            tc.tile_pool(name="ps_out", bufs=2, space="PSUM") as ps_out,
        ):
            # ---- constants into SBUF ----
            # DMA execution round-robins between the SP and ACT HWDGE rings,
            # so interleave by priority: SP gets wq + the q/k streams, ACT
            # gets wk/cos/sin/j/wv then the late consts.
            wq_sb = cpool.tile([128, NE, HD], BF16, tag="wq")
            nc.sync.dma_start(wq_sb[:], wq)
            wk_sb = cpool.tile([128, NE, HD], BF16, tag="wk")
            nc.scalar.dma_start(wk_sb[:], wk)
            cos_sb = cpool.tile([128, S], BF16, tag="cos")
            nc.scalar.dma_start(cos_sb[:], cosT)
            sin_sb = cpool.tile([128, S], BF16, tag="sin")
            nc.scalar.dma_start(sin_sb[:], sinT)
            j_sb = cpool.tile([128, 128], BF16, tag="j")
            nc.scalar.dma_start(j_sb[:], jmat)
            wv_sb = cpool.tile([128, NA, HD], BF16, tag="wv")
            nc.scalar.dma_start(wv_sb[:], wv)
            wo_sb = cpool.tile([128, D], BF16, tag="wo")
            nc.scalar.dma_start(wo_sb[:], wo)
            ones_sb = cpool.tile([128, 1], BF16, tag="ones")
            nc.scalar.dma_start(ones_sb[:], onesv)
            tri_sb = cpool.tile([128, 128], BF16, tag="tri")
            nc.scalar.dma_start(tri_sb[:], tri)
            vt_sb = cpool.tile([128, NA, S], BF16, tag="vt")

            recip_sb = spool.tile([128, NT], F32, tag="recip")
            onesf = spool.tile([1, 1], F32, tag="onesf")
            nc.gpsimd.memset(onesf[:], 1.0)
            qrope = rpool.tile([128, S], BF16, tag="qrope")
            krope = rpool.tile([128, S], BF16, tag="krope")
            vmid = rpool.tile([128, S], BF16, tag="vmid")
            attn_sb = rpool.tile([128, S], BF16, tag="attn")


            def proj_rope(src, w_sb, pool, rope_dst, s0, w):
                """Project seq cols [s0, s0+w) of q or k and apply rope."""
                cs = slice(s0, s0 + w)
                st = pool.tile([128, NE, SC], BF16, tag="stream")
                # split in thirds so the first matmuls start earlier
                h = NE // 3
                view = src[:, cs].rearrange("(e p) s -> p e s", p=128)
                nc.sync.dma_start(st[:, :h, :w], view[:, :h])
                nc.sync.dma_start(st[:, h : 2 * h, :w], view[:, h : 2 * h])
                nc.sync.dma_start(st[:, 2 * h :, :w], view[:, 2 * h :])
                ps = ps_mm.tile([128, SC], F32, tag="mm")
                for e in range(NE):
                    nc.tensor.matmul(
                        ps[:, :w],
                        w_sb[:, e, :],
                        st[:, e, :w],
                        start=(e == 0),
                        stop=(e == NE - 1),
                    )
                raw = rawpool.tile([128, SC], BF16, tag="raw")
                nc.vector.tensor_copy(raw[:, :w], ps[:, :w])
                psj = ps_mm.tile([128, SC], F32, tag="mm")
                nc.tensor.matmul(psj[:, :w], j_sb[:], raw[:, :w], start=True, stop=True)
                t1 = tmppool.tile([128, SC], BF16, tag="t1")
                nc.vector.tensor_mul(t1[:, :w], raw[:, :w], cos_sb[:, cs])
                jraw = rawpool.tile([128, SC], BF16, tag="jraw")
                nc.vector.tensor_copy(jraw[:, :w], psj[:, :w])
                t2 = tmppool.tile([128, SC], BF16, tag="t2")
                nc.vector.tensor_mul(t2[:, :w], jraw[:, :w], sin_sb[:, cs])
                nc.vector.tensor_add(rope_dst[:, cs], t1[:, :w], t2[:, :w])

            def emit_proj(s0, w):
                proj_rope(qT, wq_sb, qpool, qrope, s0, w)
                proj_rope(kT, wk_sb, kpool, krope, s0, w)
                cs = slice(s0, s0 + w)
                nc.sync.dma_start(vt_sb[:, :, cs], vT[:, :, cs])
                for t in range(s0 // 128, (s0 + w) // 128):
                    ts = slice(t * 128, (t + 1) * 128)
                    psv = ps_mm.tile([128, 128], F32, tag="mm")
                    for a in range(NA):
                        nc.tensor.matmul(
                            psv[:],
                            vt_sb[:, a, ts],
                            wv_sb[:, a, :],
                            start=(a == 0),
                            stop=(a == NA - 1),
                        )
                    nc.scalar.copy(vmid[:, ts], psv[:])

            def emit_attention(s0, w, copy_eng_seq, last=False):
                cs = slice(s0, s0 + w)
                pat = ps_at.tile([128, SC], F32, tag="at")
                psm = ps_sum.tile([1, SC], F32, tag="sm")
                ntile = (s0 + w) // 128
                for t in range(ntile):
                    col0 = max(0, t * 128 - s0)
                    cols = slice(col0, w)
                    qs = slice(s0 + col0, s0 + w)
                    ts = slice(t * 128, (t + 1) * 128)
                    # last chunk: projections are done, so scores can also
                    # rotate through the idle ps_mm banks (4-deep lookahead)
                    if last:
                        psc = (ps_mm if t % 2 else ps_sc).tile(
                            [128, SC], F32, tag="mm" if t % 2 else "sc"
                        )
                    else:
                        psc = ps_sc.tile([128, SC], F32, tag="sc")
                    nc.tensor.matmul(
                        psc[:, cols], krope[:, ts], qrope[:, qs], start=True, stop=True
                    )
                    pr = prpool.tile([128, SC], BF16, tag="pr")
                    nc.scalar.activation(
                        pr[:, cols], psc[:, cols], EXP, scale=inv_sqrt_d
                    )
                    if t * 128 >= s0:
                        nc.vector.tensor_mul(
                            pr[:, col0 : col0 + 128],
                            pr[:, col0 : col0 + 128],
                            tri_sb[:],
                        )
                    nc.tensor.matmul(
                        pat[:, cols],
                        vmid[:, ts],
                        pr[:, cols],
                        start=(t == 0),
                        stop=(t == ntile - 1),
                    )
                    nc.tensor.matmul(
                        psm[:, cols],
                        ones_sb[:],
                        pr[:, cols],
                        start=(t == 0),
                        stop=(t == ntile - 1),
                    )
                # plain attn evacuation (frees pat fast); normalization is
                # applied per-partition during the out-copies below
                nc.vector.tensor_copy(attn_sb[:, cs], pat[:, :w])
                # transpose the [1,w] sums row into per-partition layout via
                # tiny K=1 N=1 matmuls, then reciprocal
                srow = npool.tile([1, SC], F32, tag="srow")
                nc.scalar.copy(srow[:, :w], psm[:, :w])
                nt0 = s0 // 128
                ntw = w // 128
                rec_ps = ps_sum.tile([128, 4], F32, tag="sm")
                for i in range(ntw):
                    nc.tensor.matmul(
                        rec_ps[:, i : i + 1],
                        srow[0:1, i * 128 : (i + 1) * 128],
                        onesf[:],
                        start=True,
                        stop=True,
                    )
                nc.vector.reciprocal(
                    recip_sb[:, nt0 : nt0 + ntw], rec_ps[:, :ntw]
                )
                # out-projection with fused normalization
                for tt in range(nt0, nt0 + ntw):
                    ts = slice(tt * 128, (tt + 1) * 128)
                    osb = opool.tile([128, D], OUT_DTYPE, tag="osb")
                    osb_tiles.append((ts, osb, s0 + w == S))
                    for ic in range(NIC):
                        ics = slice(ic * SC, (ic + 1) * SC)
                        # the last chunk's projections are long done, so its
                        # out-proj can also rotate through the idle ps_mm
                        # slots -> 4 banks, no copy-paced PE stalls in the tail
                        if last:
                            po = (ps_mm if ic % 2 else ps_out).tile(
                                [128, SC], F32, tag="mm" if ic % 2 else "po"
                            )
                        else:
                            po = ps_out.tile([128, SC], F32, tag="po")
                        nc.tensor.matmul(
                            po[:], attn_sb[:, ts], wo_sb[:, ics], start=True, stop=True
                        )
                        use_act = next(copy_eng_seq) if copy_eng_seq else False
                        if use_act:
                            nc.scalar.activation(
                                osb[:, ics], po[:], COPY,
                                scale=recip_sb[:, tt : tt + 1],
                            )
                        else:
                            nc.vector.tensor_scalar_mul(
                                osb[:, ics], po[:], recip_sb[:, tt : tt + 1]
                            )

            def alternator():
                i = 0
                while True:
                    yield i % 2 == 0
                    i += 1

            CHUNKS = [(0, 512), (512, 512), (1024, 512), (1536, 512)]
            osb_tiles = []
            for rep in range(reps):
                ceng = alternator()
                for ci in range(len(CHUNKS)):
                    emit_proj(*CHUNKS[ci])
                    emit_attention(
                        *CHUNKS[ci],
                        copy_eng_seq=ceng,
                        last=(ci == len(CHUNKS) - 1),
                    )

                # All output writes go at the END of the SP ring: the HWDGE
                # FIFO then naturally defers them behind every input stream,
                # so they never steal bandwidth from input prefetch.
                for ts, osb, split in osb_tiles:
                    nc.sync.dma_start(out[ts, :], osb[:])
                osb_tiles.clear()

    nc.compile()
    return nc


_NC_CACHE = {}


def _get_program(reps=1):
    if reps not in _NC_CACHE:
        _NC_CACHE[reps] = _build_program(reps)
    return _NC_CACHE[reps]


def _rope_tables():
    freqs = 1.0 / (ROPE_BASE ** (np.arange(0, HD, 2, dtype=np.float64) / HD))
    ang = np.arange(S, dtype=np.float64)[:, None] * freqs[None, :]  # [S, 64]
    cos = np.cos(ang).T  # [64, S]
    sin = np.sin(ang).T
    cosT = np.concatenate([cos, cos], axis=0)  # [128, S]
    sinT = np.concatenate([-sin, sin], axis=0)  # sign-folded
    return cosT.astype(np.float32), sinT.astype(np.float32)


def _host_prep(q, k, v, W_q, W_k, W_v, W_o):
    """Build the 8 per-core input maps. Core (b, h) -> index 4*b + h."""
    # rotate-half permutation of the head dim: y[i]=x[2i], y[64+i]=x[2i+1]
    perm = np.concatenate([np.arange(0, HD, 2), np.arange(1, HD, 2)])
    cosT, sinT = _rope_tables()
    jm = np.zeros((128, 128), np.float32)
    jm[np.arange(128), (np.arange(128) + 64) % 128] = 1.0
    ones = np.ones((128, 1), np.float32)
    tri = (np.arange(128)[:, None] <= np.arange(128)[None, :]).astype(np.float32)

    common = {
        "cosT": cosT.astype(BF),
        "sinT": sinT.astype(BF),
        "jmat": jm.astype(BF),
        "onesv": ones.astype(BF),
        "tri": tri.astype(BF),
    }

    qT = [np.ascontiguousarray(q[b].T).astype(BF) for b in range(B)]
    kT = [np.ascontiguousarray(k[b].T).astype(BF) for b in range(B)]
    # v aug-slice, transposed then pre-arranged to (p, a, s)
    vT = [
        np.ascontiguousarray(
            v[b, :, D_AUG : 2 * D_AUG].T.reshape(NA, 128, S).transpose(1, 0, 2)
        ).astype(BF)
        for b in range(B)
    ]

    in_maps = []
    for core in range(8):
        b, h = divmod(core, N_HEADS_LIVE)
        hs = slice(h * HD, (h + 1) * HD)
        wq_h = W_q[hs, :][perm, :]  # [128, 1536], P-permuted rows
        wk_h = W_k[hs, :][perm, :]
        wv_h = W_v[hs, :]  # [128, 512]
        wo_h = W_o[:, hs]  # [1536, 128]

        def arrange_w(w_t, nch):  # [K, 128] -> (p, e, m) = [128, nch, 128]
            return np.ascontiguousarray(
                w_t.reshape(nch, 128, HD).transpose(1, 0, 2)
            ).astype(BF)

        in_maps.append(
            {
                "qT": qT[b],
                "kT": kT[b],
                "vT": vT[b],
                "wq": arrange_w(wq_h.T, NE),
                "wk": arrange_w(wk_h.T, NE),
                "wv": arrange_w(wv_h.T, NA),
                "wo": np.ascontiguousarray(wo_h.T).astype(BF),
                **common,
            }
        )
    return in_maps


def kernel(q, k, v, W_q, W_k, W_v, W_o):
    q = np.asarray(q, np.float32)
    k = np.asarray(k, np.float32)
    v = np.asarray(v, np.float32)
    W_q = np.asarray(W_q, np.float32)
    W_k = np.asarray(W_k, np.float32)
    W_v = np.asarray(W_v, np.float32)
    W_o = np.asarray(W_o, np.float32)

    nc = _get_program()
    in_maps = _host_prep(q, k, v, W_q, W_k, W_v, W_o)
    res = bass_utils.run_bass_kernel_spmd(nc, in_maps, core_ids=list(range(8)))

    out = np.zeros((B, S, D), np.float32)
    for core in range(8):
        b = core // N_HEADS_LIVE
        out[b] += res.results[core]["out"].astype(np.float32)
    return out


if __name__ == "__main__":
    rng = np.random.default_rng(0)
    ins = {
        "q": rng.standard_normal((B, S, D), np.float32),
        "k": rng.standard_normal((B, S, D), np.float32),
        "v": rng.standard_normal((B, S, D), np.float32),
        "W_q": rng.standard_normal((D, D), np.float32) * 0.02,
        "W_k": rng.standard_normal((D, D), np.float32) * 0.02,
        "W_v": rng.standard_normal((D_AUG, D_AUG), np.float32) * 0.02,
        "W_o": rng.standard_normal((D, D), np.float32) * 0.02,
    }
    o = kernel(**ins)
    print(o.shape, o.dtype, float(np.abs(o).max()))
